# revision 11
# baseline (speedup 1.0000x reference)
"""Trainium2 Bass kernel for NonLinearSelfAttention.

Computes, per batch b:
    S    = x_b @ x_b.T * C**-0.5          [N, N]
    P    = softmax(S, axis=-1)
    out  = (P @ x_b) @ W.T + bias         [N, OUT]

Sharding: batch-data-parallel, one batch per NeuronCore (8 cores).

Per-core algorithm (N=4096, C=128), v2 -- symmetric-exp + fp8 S:
  - E = exp(scale*S) is symmetric: only upper-triangle tiles (r<=j) go
    through the scalar engine's exp (halves ACT work, the baseline
    bottleneck).  Mirror tiles E[j,r] = E[r,j]^T come from the DMA
    transpose XBAR (SBUF->SBUF, bf16) -- no PE or DVE cycles.
  - S matmuls run in fp8 (e4m3) DoubleRow mode: contraction padded
    from 128 to 256 with a zero k-tile, 4x fewer PE cycles than bf16.
  - The S diagonal is destroyed by fp8 quantization noise, so it is
    killed in-matmul (the diag pair's second k-tile points at
    -240I/+240I blocks, accumulating -57600 on the diagonal: exp -> 0)
    and re-added exactly in the epilogue from d_i = scale*||x_i||^2
    computed in fp32 on the vector engine.
  - The Linear folds through the attention: y = (E @ [z | 1]) / r + bias
    with z = x @ W.T.  The ones column produces softmax row-sums in
    per-partition layout for free (r_i = sum_j E[j,i] = sum_j E[i,j]).
  - Row-band schedule: after band b (rows 4b..4b+3) is exp'd, output
    quad b's AV matmuls have all lhsT tiles (directs from earlier
    bands' upper tiles, mirrors transposed within band b); they are
    emitted interleaved with band b+1's S chunks to keep PE fed while
    the scalar engine works.
"""
import numpy as np

import concourse.bass as bass
import concourse.tile as tile
from concourse.masks import make_identity
from concourse import bacc, mybir
from concourse import bass_utils

B = 8          # batches = cores
N = 4096       # sequence length
C = 128        # feature dim
OUT = 128      # linear out dim
NT = N // 128  # 32 tiles
SCALE = float(C) ** -0.5
CH = 8         # j-tiles per S chunk (one PSUM buffer = 2 banks)

F32 = mybir.dt.float32
BF16 = mybir.dt.bfloat16
FP8 = mybir.dt.float8e4

# fp8 arena layout (elements per partition)
XT8 = 0            # [0, 4096): xT in fp8
ZOFF = N           # [4096, 4224): zeros k-tile
NEGI = N + 128     # [4224, 4352): -240 * I
KILLR = N + 256    # [4352, 4608): [240*I | 0]
AW = N + 512


class _Arena:
    """First-fit interval allocator (element granularity) for the E store."""

    def __init__(self, total):
        self.total = total
        self.free = [(0, total)]
        self.used = 0
        self.peak = 0

    def alloc(self, size):
        for i, (o, s) in enumerate(self.free):
            if s >= size:
                if s == size:
                    del self.free[i]
                else:
                    self.free[i] = (o + size, s - size)
                self.used += size
                self.peak = max(self.peak, self.used)
                return o
        raise MemoryError(f"arena exhausted: want {size}, free {self.free}")

    def release(self, off, size):
        self.used -= size
        self.free.append((off, size))
        self.free.sort()
        merged = []
        for o, s in self.free:
            if merged and merged[-1][0] + merged[-1][1] == o:
                merged[-1] = (merged[-1][0], merged[-1][1] + s)
            else:
                merged.append((o, s))
        self.free = [tuple(t) for t in merged]


def _chunks_of_row(r):
    """S chunk grid for row-tile r: (j0, jlo, hi) per chunk."""
    out = []
    for j0 in range(0, NT, CH):
        hi = min(j0 + CH, NT)
        jlo = max(r, j0)
        if jlo < hi:
            out.append((j0, jlo, hi))
    return out


def _build():
    nc = bacc.Bacc("TRN2", target_bir_lowering=False, debug=False, num_devices=B)
    x_d = nc.dram_tensor("x", [N, C], F32, kind="ExternalInput").ap()
    w_d = nc.dram_tensor("W", [OUT, C], F32, kind="ExternalInput").ap()
    b_d = nc.dram_tensor("b", [OUT], F32, kind="ExternalInput").ap()
    o_d = nc.dram_tensor("out", [N, OUT], F32, kind="ExternalOutput").ap()

    # --- allocation schedule for the E stores (directs / mirrors) ---
    def _sched(arena_d, arena_m, emit=None, band_end=None):
        dfree = {}   # quad -> [(off, size)] direct half-chunk frees
        mfree = {}   # quad -> [(off, size)] mirror run frees
        for b in range(8):
            for r in range(4 * b, 4 * b + 4):
                for j0, jlo, hi in _chunks_of_row(r):
                    off = arena_d.alloc((hi - jlo) * 128)
                    for h0 in range(j0, hi, 4):
                        h_lo, h_hi = max(jlo, h0), min(hi, h0 + 4)
                        if h_lo < h_hi:
                            dfree.setdefault((h0 + 3) // 4, []).append(
                                (off + (h_lo - jlo) * 128, (h_hi - h_lo) * 128))
                    mlo = max(r + 1, j0)
                    moff = arena_m.alloc((hi - mlo) * 128) if mlo < hi else None
                    if moff is not None:
                        mfree.setdefault(r // 4, []).append(
                            (moff, (hi - mlo) * 128))
                    if emit:
                        emit(b, r, j0, jlo, hi, off, moff)
            if band_end:
                band_end(b)
            for o, s in dfree.pop(b - 1, []):
                arena_d.release(o, s)
            for o, s in mfree.pop(b - 1, []):
                arena_m.release(o, s)

    simd, simm = _Arena(1 << 30), _Arena(1 << 30)
    _sched(simd, simm)
    DW = simd.peak + 5120
    MW = simm.peak + 512

    chunks_in_band = [sum(len(_chunks_of_row(r)) for r in range(4 * b, 4 * b + 4))
                      for b in range(8)]

    with tile.TileContext(nc) as tc:
        with tc.tile_pool(name="const", bufs=1) as const, \
             tc.tile_pool(name="xw", bufs=2) as xw, \
             tc.tile_pool(name="ywork", bufs=2) as ywork, \
             tc.tile_pool(name="ps_s", bufs=2, space="PSUM") as ps_s, \
             tc.tile_pool(name="ps_acc", bufs=2, space="PSUM") as ps_acc:

            # ---------------- setup ----------------
            x_view = x_d.rearrange("(t p) c -> p t c", p=128)
            w_sb = const.tile([128, 128], F32)
            nc.sync.dma_start(w_sb, w_d)
            bias_bc = const.tile([128, 128], F32)
            nc.sync.dma_start(bias_bc, bass.AP(tensor=b_d.tensor, offset=b_d.offset,
                                               ap=[[0, 128]] + b_d.ap))
            w_bf = const.tile([128, 128], BF16)
            nc.vector.tensor_copy(w_bf, w_sb)
            wT = const.tile([128, 128], BF16)
            nc.sync.dma_start(wT, w_bf, transpose=True)

            # stream x in 8-tile slabs: bf16 cast, DMA-transpose, d = ||x||^2
            xT_bf = const.tile([128, NT, 128], BF16)
            d_sb = const.tile([128, NT], F32)
            sq = const.tile([128, 8, 128], F32)
            for t0 in range(0, NT, 8):
                xs = xw.tile([128, 8, 128], F32, name="xs", tag="xs")
                nc.sync.dma_start(xs, x_view[:, t0:t0 + 8, :])
                xb = xw.tile([128, 8, 128], BF16, name="xb", tag="xb")
                nc.vector.tensor_copy(xb, xs)
                nc.sync.dma_start(
                    xT_bf[:, t0:t0 + 8, :],
                    xb.rearrange("p a b -> p (a b)"), transpose=True)
                nc.vector.tensor_tensor(sq, xs, xs, mybir.AluOpType.mult)
                nc.vector.tensor_reduce(d_sb[:, t0:t0 + 8], sq,
                                        mybir.AxisListType.X,
                                        mybir.AluOpType.add)
            eii = const.tile([128, NT], F32)
            nc.scalar.activation(eii, d_sb, mybir.ActivationFunctionType.Exp,
                                 scale=SCALE)

            # fp8 arena: xT8 + zeros + kill blocks
            a8 = const.tile([128, AW], FP8)
            nc.vector.tensor_copy(a8[:, 0:N], xT_bf.rearrange("p a b -> p (a b)"))
            nc.vector.memset(a8[:, ZOFF:AW], 0.0)
            ident = const.tile([128, 128], BF16)
            make_identity(nc, ident)
            nc.vector.tensor_scalar(a8[:, NEGI:NEGI + 128], ident, -240.0, None,
                                    mybir.AluOpType.mult)
            nc.vector.tensor_scalar(a8[:, KILLR:KILLR + 128], ident, 240.0, None,
                                    mybir.AluOpType.mult)

            # z~ = [x @ W.T | 1] (bf16)
            zt = const.tile([128, NT, 129], BF16)
            nc.vector.memset(zt[:, :, 128], 1.0)
            for g in range(NT // 4):
                z_ps = ps_s.tile([128, CH * 128], F32, name="s_ps",
                                 tag="s")[:, 0:512]
                for u in range(4):
                    j = g * 4 + u
                    nc.tensor.matmul(z_ps[:, u * 128:(u + 1) * 128],
                                     xT_bf[:, j, :], wT, start=True, stop=True)
                nc.vector.tensor_copy(
                    zt[:, g * 4:(g + 1) * 4, 0:128],
                    z_ps.rearrange("p (j c) -> p j c", c=128))

            zeros128 = const.tile([128, 128], BF16)
            nc.vector.memset(zeros128, 0.0)
            dummy258 = const.tile([128, 258], BF16)
            nc.vector.memset(dummy258, 0.0)

            # E stores: directs (upper tiles) and mirrors (transposes)
            est_d = const.tile([128, DW], BF16)
            est_m = const.tile([128, MW], BF16)
            slot = {}        # (jt, it) -> (tile, element offset)

            def a8_ap(col, width, delta):
                return bass.AP(tensor=a8.tensor, offset=a8.offset + col,
                               ap=[a8.ap[0], [delta, 2], [1, width]])

            o_view = o_d.rearrange("(t p) c -> p t c", p=128)

            def av_quad_tasks(q):
                """Emit-callables: 2 bank-openers, then per out-block R:
                32 matmuls (grouped 8) + epilogue."""
                accs = [ps_acc.tile([128, 258], F32, name=f"acc{p}", tag=f"a{p}")
                        for p in range(2)]

                def acc_slice(k, w=129):
                    return accs[k // 2][:, (k % 2) * 129:(k % 2) * 129 + w]

                tasks = []
                for p in range(2):
                    def opener(p=p):
                        nc.tensor.matmul(accs[p], zeros128, dummy258,
                                         start=True, stop=False,
                                         skip_group_check=True)
                    tasks.append(opener)
                y4 = ywork.tile([128, 4, 128], F32, name="y4", tag="y4")
                for k in range(4):
                    R = 4 * q + k
                    for jt0 in range(0, NT, 8):
                        def mm(k=k, R=R, jt0=jt0):
                            for jt in range(jt0, jt0 + 8):
                                t, eoff = slot[(jt, R)]
                                nc.tensor.matmul(
                                    acc_slice(k), t[:, eoff:eoff + 128],
                                    zt[:, jt, :], start=False,
                                    stop=(jt == NT - 1),
                                    skip_group_check=True)
                        tasks.append(mm)

                    def epi(k=k, R=R, y4=y4):
                        num = ywork.tile([128, 128], F32, name="num", tag="num")
                        nc.vector.scalar_tensor_tensor(
                            num, zt[:, R, 0:128], eii[:, R:R + 1],
                            acc_slice(k, 128),
                            op0=mybir.AluOpType.mult, op1=mybir.AluOpType.add)
                        rtot = ywork.tile([128, 1], F32, name="rt", tag="rt")
                        nc.vector.tensor_tensor(rtot, acc_slice(k, 129)[:, 128:129],
                                                eii[:, R:R + 1],
                                                mybir.AluOpType.add)
                        rinv = ywork.tile([128, 1], F32, name="ri", tag="ri")
                        nc.vector.reciprocal(rinv, rtot)
                        nc.vector.scalar_tensor_tensor(
                            y4[:, k, :], num, rinv, bias_bc,
                            op0=mybir.AluOpType.mult, op1=mybir.AluOpType.add)
                        if k == 3:
                            nc.sync.dma_start(o_view[:, q * 4:(q + 1) * 4, :], y4)
                    tasks.append(epi)
                return tasks

            # ---------------- main schedule ----------------
            state = {"pending": [], "i": 0, "n": 0}

            def emit(b, r, j0, jlo, hi, off, moff):
                if state["n"] == 0:
                    state["i"], state["n"] = 0, chunks_in_band[b]
                ntile = hi - jlo
                s_ps = ps_s.tile([128, CH * 128], F32, name="s_ps", tag="s")
                j = jlo
                while j < hi:
                    w2 = 2 if j + 1 < hi else 1
                    col = j * 128
                    is_diag = (j == r)
                    lhsT = a8_ap(r * 128, 128,
                                 (NEGI if is_diag else ZOFF) - r * 128)
                    rhs = a8_ap(col, 128 * w2,
                                (KILLR - col) if is_diag else 0)
                    nc.tensor.matmul(
                        s_ps[:, (j - j0) * 128:(j - j0 + w2) * 128],
                        lhsT, rhs, start=True, stop=True,
                        perf_mode=mybir.MatmulPerfMode.DoubleRow)
                    j += w2
                for k in range(ntile):
                    slot[(r, jlo + k)] = (est_d, off + k * 128)
                nc.scalar.activation(
                    est_d[:, off:off + ntile * 128],
                    s_ps[:, (jlo - j0) * 128:(hi - j0) * 128],
                    mybir.ActivationFunctionType.Exp, scale=SCALE)
                mlo = max(r + 1, j0)
                nt2 = hi - mlo
                if nt2 > 0:
                    for k in range(nt2):
                        slot[(mlo + k, r)] = (est_m, moff + k * 128)
                    src = est_d[:, off + (mlo - jlo) * 128:off + (hi - jlo) * 128]
                    dst = est_m[:, moff:moff + nt2 * 128].rearrange(
                        "p (m f) -> p m f", f=128)
                    nc.sync.dma_start(dst, src, transpose=True)
                # interleave pending AV work of the previous quad
                state["i"] += 1
                pend = state["pending"]
                if pend:
                    remaining = state["n"] - state["i"]
                    k = (len(pend) if remaining == 0
                         else max(1, len(pend) // (remaining + 1)))
                    for _ in range(min(k, len(pend))):
                        pend.pop(0)()
                if state["i"] == state["n"]:
                    state["n"] = 0

            def band_end(b):
                for t in state["pending"]:
                    t()
                state["pending"] = av_quad_tasks(b)

            arena_d, arena_m = _Arena(DW), _Arena(MW)
            _sched(arena_d, arena_m, emit=emit, band_end=band_end)
            for t in state["pending"]:
                t()

    nc.compile()
    return nc


_NC_CACHE = {}


def _get_nc():
    if "nc" not in _NC_CACHE:
        _NC_CACHE["nc"] = _build()
    return _NC_CACHE["nc"]


def kernel(x, W, b, _trace=False):
    """x: [8, 4096, 128] f32, W: [128, 128] f32, b: [128] f32 -> [8, 4096, 128] f32."""
    nc = _get_nc()
    x = np.ascontiguousarray(np.asarray(x, dtype=np.float32))
    W = np.ascontiguousarray(np.asarray(W, dtype=np.float32))
    b = np.ascontiguousarray(np.asarray(b, dtype=np.float32))
    in_maps = [{"x": x[i], "W": W, "b": b} for i in range(B)]
    res = bass_utils.run_bass_kernel_spmd(nc, in_maps, core_ids=list(range(B)),
                                          trace=_trace)
    out = np.stack([r["out"] for r in res.results]).astype(np.float32)
    if _trace:
        return out, res
    return out


# revision 29
# speedup vs baseline: 1.0666x; 1.0666x over previous
"""Trainium2 Bass kernel for NonLinearSelfAttention.

Computes, per batch b:
    S    = x_b @ x_b.T * C**-0.5          [N, N]
    P    = softmax(S, axis=-1)
    out  = (P @ x_b) @ W.T + bias         [N, OUT]

Sharding: batch-data-parallel, one batch per NeuronCore (8 cores).

Per-core algorithm (N=4096, C=128):
  - E = exp(scale*S) is symmetric, so the tile E[J-block, A-block] computed in
    [j, i] layout is directly the lhsT needed by the P@V matmul for output
    block A — no transposes in the main loop.
  - The Linear folds through the attention: y = (E @ [z | 1]) / r + bias with
    z = x @ W.T, because (P x) W.T = P (x W.T).  The appended ones column
    produces the softmax row-sums r in per-partition layout for free
    (r_i = sum_j E[j, i] = sum_j E[i, j] by symmetry).
  - No max-subtraction needed: logits are ~N(0,1) with diagonal ~ sqrt(C)+,
    max ~ 20, exp(20) is well inside fp32 range.
"""
import numpy as np

import concourse.bass as bass
import concourse.tile as tile
from concourse.masks import make_identity
from concourse import bacc, mybir
from concourse import bass_utils

B = 8          # batches = cores
N = 4096       # sequence length
C = 128        # feature dim
OUT = 128      # linear out dim
NT = N // 128  # 32 j-tiles
QW = 512       # i-columns processed per quad-block
NQ = N // QW   # 8 quad blocks
SCALE = float(C) ** -0.5

F32 = mybir.dt.float32
F32R = mybir.dt.float32r
BF16 = mybir.dt.bfloat16
FP8 = mybir.dt.float8e4

# fp8 arena layout (elements per partition): xT8, zero k-tile, kill blocks.
# The S matmuls run in fp8 DoubleRow mode (contraction 128 padded to 256
# with a zero k-tile).  The S diagonal is poisoned by fp8 noise, so the
# diag pair's second k-tile points at -240I x [240I|0] (or [0|240I]),
# accumulating -57600 on the diagonal: exp -> 0; the exact diagonal
# E_ii = exp(scale*||x_i||^2) is re-added in the epilogue from fp32 x.
ZOFF = N           # [4096, 4224): zeros k-tile
NEGI = N + 128     # [4224, 4352): -240 * I
KILLA = N + 256    # [4352, 4608): [240*I | 0]
KILLB = N + 512    # [4608, 4864): [0 | 240*I]
AW = N + 768


def _build(ctx_dtype=BF16):
    nc = bacc.Bacc("TRN2", target_bir_lowering=False, debug=False, num_devices=B)
    x_d = nc.dram_tensor("x", [N, C], F32, kind="ExternalInput").ap()
    w_d = nc.dram_tensor("W", [OUT, C], F32, kind="ExternalInput").ap()
    b_d = nc.dram_tensor("b", [OUT], F32, kind="ExternalInput").ap()
    o_d = nc.dram_tensor("out", [N, OUT], F32, kind="ExternalOutput").ap()

    with tile.TileContext(nc) as tc:
        with tc.tile_pool(name="const", bufs=1) as const, \
             tc.tile_pool(name="bwork", bufs=6) as bwork, \
             tc.tile_pool(name="ywork", bufs=2) as ywork, \
             tc.tile_pool(name="ps_work", bufs=2, space="PSUM") as ps_work, \
             tc.tile_pool(name="ps_acc", bufs=2, space="PSUM") as ps_acc:

            # ---- setup ----
            # x loads: few big DMAs (each InstDMACopy splits across all 16
            # SDMA slots; many small DMAs pay ~600ns serial issue each)
            x_nat = const.tile([128, NT, 128], F32)       # x tiles [j within tile, c]
            x_view = x_d.rearrange("(t p) c -> p t c", p=128)
            # small leading chunks so the cast/transpose pipeline starts early
            bounds = [0, 4, 8, 16, 24, NT]
            for lo, hi in zip(bounds, bounds[1:]):
                nc.sync.dma_start(x_nat[:, lo:hi, :], x_view[:, lo:hi, :])

            w_sb = const.tile([128, 128], F32)            # W [o, c]
            nc.sync.dma_start(w_sb, w_d)
            bias_bc = const.tile([128, 128], F32)         # bias broadcast to all partitions
            nc.sync.dma_start(bias_bc, bass.AP(tensor=b_d.tensor, offset=b_d.offset,
                                               ap=[[0, 128]] + b_d.ap))

            # cast to bf16, then PE-transpose (bf16: single matmul per tile;
            # fp32 transposes lower to 2).  PSUM->SBUF casts ride the scalar
            # engine, idle during setup.
            x_bf = const.tile([128, NT, 128], BF16)
            for lo, hi in zip(bounds, bounds[1:]):
                nc.vector.tensor_copy(x_bf[:, lo:hi, :], x_nat[:, lo:hi, :])
            w_bf = const.tile([128, 128], BF16)
            nc.vector.tensor_copy(w_bf, w_sb)

            ident = const.tile([128, 128], BF16)
            make_identity(nc, ident)
            xT = const.tile([128, N], BF16)               # [c, n]

            def emit_xT_group(g):
                t_ps = ps_work.tile([128, 512], BF16, name="t_ps", tag="pswork")
                for u in range(4):
                    nc.tensor.transpose(t_ps[:, u * 128:(u + 1) * 128],
                                        x_bf[:, g * 4 + u, :], ident)
                nc.vector.tensor_copy(xT[:, g * 512:(g + 1) * 512], t_ps)

            xT_state = {"emitted": 0}

            def ensure_xT(j_hi):
                need = min(NT // 4, max(1, (j_hi + 3) // 4))
                while xT_state["emitted"] < need:
                    emit_xT_group(xT_state["emitted"])
                    xT_state["emitted"] += 1

            ensure_xT(4)  # group 0: quad 0's rhs columns
            wt_ps = ps_work.tile([128, 512], BF16, name="t_ps", tag="pswork")
            nc.tensor.transpose(wt_ps[:, 0:128], w_bf, ident)
            wT = const.tile([128, 128], BF16)             # wT[c, o] = W[o, c]
            nc.vector.tensor_copy(wT, wt_ps[:, 0:128])

            # d_i = ||x_i||^2 in fp32; E_ii = exp(scale * d_i)
            d_sb = const.tile([128, NT], F32)
            sqs = const.tile([128, 128], F32)
            for t in range(NT):
                nc.vector.tensor_tensor(sqs, x_nat[:, t, :], x_nat[:, t, :],
                                        mybir.AluOpType.mult)
                nc.vector.tensor_reduce(d_sb[:, t:t + 1], sqs,
                                        mybir.AxisListType.X,
                                        mybir.AluOpType.add)
            eii = const.tile([128, NT], F32)
            nc.scalar.activation(eii, d_sb, mybir.ActivationFunctionType.Exp,
                                 scale=SCALE)

            # fp8 arena for the DoubleRow S matmuls
            a8 = const.tile([128, AW], FP8)
            a8_state = {"filled": 0}

            def ensure_a8(j_hi):
                # cast xT -> fp8 in 8-tile pieces as xT groups land
                need = min(NT, max(0, j_hi))
                while a8_state["filled"] < need:
                    g = a8_state["filled"]
                    g2 = min(NT, g + 8)
                    ensure_xT(g2)
                    nc.vector.tensor_copy(a8[:, g * 128:g2 * 128],
                                          xT[:, g * 128:g2 * 128])
                    a8_state["filled"] = g2

            nc.vector.memset(a8[:, ZOFF:NEGI], 0.0)
            nc.vector.tensor_scalar(a8[:, NEGI:KILLA], ident, -240.0, None,
                                    mybir.AluOpType.mult)
            nc.vector.memset(a8[:, KILLA:AW], 0.0)
            nc.vector.tensor_scalar(a8[:, KILLA:KILLA + 128], ident, 240.0,
                                    None, mybir.AluOpType.mult)
            nc.vector.tensor_scalar(a8[:, KILLB + 128:KILLB + 256], ident,
                                    240.0, None, mybir.AluOpType.mult)

            def a8_ap(col, width, delta):
                return bass.AP(tensor=a8.tensor, offset=a8.offset + col,
                               ap=[a8.ap[0], [delta, 2], [1, width]])

            # z~ = [x @ W.T | 1]  (bf16), tiled [j within tile, 129]
            zt = const.tile([128, NT, 129], ctx_dtype)
            nc.vector.memset(zt[:, :, 128], 1.0)

            def emit_z_group(g):
                z_ps = ps_work.tile([128, 512], F32, name="z_ps", tag="pswork")
                for u in range(4):
                    j = g * 4 + u
                    nc.tensor.matmul(z_ps[:, u * 128:(u + 1) * 128],
                                     xT[:, j * 128:(j + 1) * 128], wT,
                                     start=True, stop=True)
                nc.vector.tensor_copy(
                    zt[:, g * 4:(g + 1) * 4, 0:128],
                    z_ps.rearrange("p (j c) -> p j c", c=128))

            z_state = {"emitted": 0}

            def ensure_z(j_hi):
                need = min(NT // 4, (j_hi + 3) // 4)
                while z_state["emitted"] < need:
                    emit_z_group(z_state["emitted"])
                    z_state["emitted"] += 1

            zeros128 = const.tile([128, 128], ctx_dtype)
            nc.vector.memset(zeros128, 0.0)
            dummy258 = const.tile([128, 258], ctx_dtype)
            nc.vector.memset(dummy258, 0.0)


            # prefetch a couple of xT/z groups so quad 0's pipeline starts deep
            ensure_xT(8)
            ensure_z(4)

            # ---- main loop ----
            # exp tiles span up to 3 PSUM banks (j-block groups of 3) to
            # amortize the ~352-cycle ACTIVATE overhead.  The four acc
            # accumulators pack two-per-bank: a zero matmul opens the bank's
            # accumulation group (start=True clears has_written bank-wide),
            # then every AV matmul accumulates with start=False.
            # S-matmuls are emitted one group AHEAD so they sit in front of
            # the previous group's AV matmuls in the PE FIFO — otherwise the
            # scalar engine stalls ~1us at every quad boundary (head-of-line
            # blocking behind AVs that wait on exp).
            JG = [2] + [3] * 10       # j-block group sizes per quad (sum=32)
            NB = QW // 128            # i-blocks per quad (4)
            groups = []
            for q in range(NQ):
                jb = 0
                for hi, gsz in enumerate(JG):
                    groups.append((q, jb, gsz, hi))
                    jb += gsz

            s_tiles = {}

            def emit_S(idx):
                q, jb, gsz, hi = groups[idx]
                ensure_a8(max(jb + gsz, 4 * q + 4))
                s_ps = ps_work.tile([128, QW * gsz], F32, name="s_ps",
                                    tag="pswork")
                for u in range(gsz):
                    j = jb + u
                    for h in range(2):
                        cols = q * 512 + h * 256
                        t0 = q * 4 + 2 * h
                        is_diag = t0 <= j <= t0 + 1
                        lhsT = a8_ap(j * 128, 128,
                                     (NEGI if is_diag else ZOFF) - j * 128)
                        if is_diag:
                            kb = KILLA if j == t0 else KILLB
                            rhs = a8_ap(cols, 256, kb - cols)
                        else:
                            rhs = a8_ap(cols, 256, 0)
                        nc.tensor.matmul(
                            s_ps[:, u * QW + h * 256:u * QW + h * 256 + 256],
                            lhsT, rhs, start=True, stop=True,
                            perf_mode=mybir.MatmulPerfMode.DoubleRow)
                s_tiles[idx] = s_ps

            emit_S(0)
            acc = None
            acc_slice = None
            for idx, (q, jb, gsz, hi) in enumerate(groups):
                if hi == 0:
                    acc = [ps_acc.tile([128, 258], F32, name=f"acc{p}",
                                       tag="acc")
                           for p in range(NB // 2)]

                    def acc_slice(k, w=129, _acc=acc):
                        return _acc[k // 2][:, (k % 2) * 129:(k % 2) * 129 + w]

                if idx + 1 < len(groups):
                    emit_S(idx + 1)
                s_ps = s_tiles.pop(idx)
                b_sb = bwork.tile([128, QW * gsz], ctx_dtype, name="b_sb",
                                  tag="b_sb")
                nc.scalar.activation(b_sb, s_ps, mybir.ActivationFunctionType.Exp,
                                     scale=SCALE)
                ensure_z(jb + gsz)
                if hi == 0:
                    for p in range(NB // 2):
                        nc.tensor.matmul(acc[p], zeros128, dummy258,
                                         start=True, stop=False,
                                         skip_group_check=True)
                for u in range(gsz):
                    j = jb + u
                    for k in range(NB):
                        nc.tensor.matmul(
                            acc_slice(k),
                            b_sb[:, u * QW + k * 128:u * QW + (k + 1) * 128],
                            zt[:, j, :], start=False, stop=(j == NT - 1),
                            skip_group_check=True)
                if hi != len(JG) - 1:
                    continue
                # epilogue: y = acc[:, :128] / acc[:, 128] + bias; one DMA/quad
                y4 = ywork.tile([128, NB, 128], F32, name="y4", tag="y4")
                for k in range(NB):
                    R = q * NB + k
                    rtot = ywork.tile([128, 1], F32, name="rtot", tag="rtot")
                    nc.vector.tensor_tensor(rtot, acc_slice(k, 129)[:, 128:129],
                                            eii[:, R:R + 1], mybir.AluOpType.add)
                    rinv = ywork.tile([128, 1], F32, name="rinv", tag="rinv")
                    nc.vector.reciprocal(rinv, rtot)
                    nc.vector.scalar_tensor_tensor(
                        y4[:, k, :], zt[:, R, 0:128], eii[:, R:R + 1],
                        acc_slice(k, 128),
                        op0=mybir.AluOpType.mult, op1=mybir.AluOpType.add)
                    nc.vector.scalar_tensor_tensor(
                        y4[:, k, :], y4[:, k, :], rinv, bias_bc,
                        op0=mybir.AluOpType.mult, op1=mybir.AluOpType.add)
                o_view = o_d.rearrange("(t p) c -> p t c", p=128)
                nc.sync.dma_start(o_view[:, q * NB:(q + 1) * NB, :], y4)

    nc.compile()
    return nc


_NC_CACHE = {}


def _get_nc():
    if "nc" not in _NC_CACHE:
        _NC_CACHE["nc"] = _build()
    return _NC_CACHE["nc"]


def kernel(x, W, b, _trace=False):
    """x: [8, 4096, 128] f32, W: [128, 128] f32, b: [128] f32 -> [8, 4096, 128] f32."""
    nc = _get_nc()
    x = np.ascontiguousarray(np.asarray(x, dtype=np.float32))
    W = np.ascontiguousarray(np.asarray(W, dtype=np.float32))
    b = np.ascontiguousarray(np.asarray(b, dtype=np.float32))
    in_maps = [{"x": x[i], "W": W, "b": b} for i in range(B)]
    res = bass_utils.run_bass_kernel_spmd(nc, in_maps, core_ids=list(range(B)),
                                          trace=_trace)
    out = np.stack([r["out"] for r in res.results]).astype(np.float32)
    if _trace:
        return out, res
    return out



# revision 30
# speedup vs baseline: 1.0701x; 1.0033x over previous
"""Trainium2 Bass kernel for NonLinearSelfAttention.

Computes, per batch b:
    S    = x_b @ x_b.T * C**-0.5          [N, N]
    P    = softmax(S, axis=-1)
    out  = (P @ x_b) @ W.T + bias         [N, OUT]

Sharding: batch-data-parallel, one batch per NeuronCore (8 cores).

Per-core algorithm (N=4096, C=128):
  - E = exp(scale*S) is symmetric, so the tile E[J-block, A-block] computed in
    [j, i] layout is directly the lhsT needed by the P@V matmul for output
    block A — no transposes in the main loop.
  - The Linear folds through the attention: y = (E @ [z | 1]) / r + bias with
    z = x @ W.T, because (P x) W.T = P (x W.T).  The appended ones column
    produces the softmax row-sums r in per-partition layout for free
    (r_i = sum_j E[j, i] = sum_j E[i, j] by symmetry).
  - No max-subtraction needed: logits are ~N(0,1) with diagonal ~ sqrt(C)+,
    max ~ 20, exp(20) is well inside fp32 range.
"""
import numpy as np

import concourse.bass as bass
import concourse.tile as tile
from concourse.masks import make_identity
from concourse import bacc, mybir
from concourse import bass_utils

B = 8          # batches = cores
N = 4096       # sequence length
C = 128        # feature dim
OUT = 128      # linear out dim
NT = N // 128  # 32 j-tiles
QW = 512       # i-columns processed per quad-block
NQ = N // QW   # 8 quad blocks
SCALE = float(C) ** -0.5

F32 = mybir.dt.float32
F32R = mybir.dt.float32r
BF16 = mybir.dt.bfloat16
FP8 = mybir.dt.float8e4

# fp8 arena layout (elements per partition): xT8, zero k-tile, kill blocks.
# The S matmuls run in fp8 DoubleRow mode (contraction 128 padded to 256
# with a zero k-tile).  The S diagonal is poisoned by fp8 noise, so the
# diag pair's second k-tile points at -240I x [240I|0] (or [0|240I]),
# accumulating -57600 on the diagonal: exp -> 0; the exact diagonal
# E_ii = exp(scale*||x_i||^2) is re-added in the epilogue from fp32 x.
ZOFF = N           # [4096, 4224): zeros k-tile
NEGI = N + 128     # [4224, 4352): -240 * I
KILLA = N + 256    # [4352, 4608): [240*I | 0]
KILLB = N + 512    # [4608, 4864): [0 | 240*I]
AW = N + 768


def _build(ctx_dtype=BF16):
    nc = bacc.Bacc("TRN2", target_bir_lowering=False, debug=False, num_devices=B)
    x_d = nc.dram_tensor("x", [N, C], F32, kind="ExternalInput").ap()
    w_d = nc.dram_tensor("W", [OUT, C], F32, kind="ExternalInput").ap()
    b_d = nc.dram_tensor("b", [OUT], F32, kind="ExternalInput").ap()
    o_d = nc.dram_tensor("out", [N, OUT], F32, kind="ExternalOutput").ap()

    with tile.TileContext(nc) as tc:
        with tc.tile_pool(name="const", bufs=1) as const, \
             tc.tile_pool(name="bwork", bufs=6) as bwork, \
             tc.tile_pool(name="ywork", bufs=2) as ywork, \
             tc.tile_pool(name="ps_work", bufs=2, space="PSUM") as ps_work, \
             tc.tile_pool(name="ps_acc", bufs=2, space="PSUM") as ps_acc:

            # ---- setup ----
            # x loads: few big DMAs (each InstDMACopy splits across all 16
            # SDMA slots; many small DMAs pay ~600ns serial issue each)
            x_nat = const.tile([128, NT, 128], F32)       # x tiles [j within tile, c]
            x_view = x_d.rearrange("(t p) c -> p t c", p=128)
            # small leading chunks so the cast/transpose pipeline starts early
            bounds = [0, 4, 8, 16, 24, NT]
            for lo, hi in zip(bounds, bounds[1:]):
                nc.sync.dma_start(x_nat[:, lo:hi, :], x_view[:, lo:hi, :])

            w_sb = const.tile([128, 128], F32)            # W [o, c]
            nc.sync.dma_start(w_sb, w_d)
            bias_bc = const.tile([128, 128], F32)         # bias broadcast to all partitions
            nc.sync.dma_start(bias_bc, bass.AP(tensor=b_d.tensor, offset=b_d.offset,
                                               ap=[[0, 128]] + b_d.ap))

            # cast to bf16, then PE-transpose (bf16: single matmul per tile;
            # fp32 transposes lower to 2).  PSUM->SBUF casts ride the scalar
            # engine, idle during setup.
            x_bf = const.tile([128, NT, 128], BF16)
            for lo, hi in zip(bounds, bounds[1:]):
                nc.vector.tensor_copy(x_bf[:, lo:hi, :], x_nat[:, lo:hi, :])
            w_bf = const.tile([128, 128], BF16)
            nc.vector.tensor_copy(w_bf, w_sb)

            ident = const.tile([128, 128], BF16)
            make_identity(nc, ident)
            xT = const.tile([128, N], BF16)               # [c, n]

            def emit_xT_group(g):
                t_ps = ps_work.tile([128, 512], BF16, name="t_ps", tag="pswork")
                for u in range(4):
                    nc.tensor.transpose(t_ps[:, u * 128:(u + 1) * 128],
                                        x_bf[:, g * 4 + u, :], ident)
                nc.vector.tensor_copy(xT[:, g * 512:(g + 1) * 512], t_ps)

            xT_state = {"emitted": 0}

            def ensure_xT(j_hi):
                need = min(NT // 4, max(1, (j_hi + 3) // 4))
                while xT_state["emitted"] < need:
                    emit_xT_group(xT_state["emitted"])
                    xT_state["emitted"] += 1

            ensure_xT(4)  # group 0: quad 0's rhs columns
            wt_ps = ps_work.tile([128, 512], BF16, name="t_ps", tag="pswork")
            nc.tensor.transpose(wt_ps[:, 0:128], w_bf, ident)
            wT = const.tile([128, 128], BF16)             # wT[c, o] = W[o, c]
            nc.vector.tensor_copy(wT, wt_ps[:, 0:128])

            # d_i = ||x_i||^2 in fp32; E_ii = exp(scale * d_i).
            # Emitted lazily per quad (epilogue-only dependency) so the
            # DVE ops don't delay the setup-critical xT/z/a8 casts.
            d_sb = const.tile([128, NT], F32)
            sqs = const.tile([128, 128], F32)
            eii = const.tile([128, NT], F32)

            def emit_eii(q):
                for t in range(4 * q, 4 * q + 4):
                    nc.vector.tensor_tensor(sqs, x_nat[:, t, :], x_nat[:, t, :],
                                            mybir.AluOpType.mult)
                    nc.vector.tensor_reduce(d_sb[:, t:t + 1], sqs,
                                            mybir.AxisListType.X,
                                            mybir.AluOpType.add)
                nc.scalar.activation(eii[:, 4 * q:4 * q + 4],
                                     d_sb[:, 4 * q:4 * q + 4],
                                     mybir.ActivationFunctionType.Exp,
                                     scale=SCALE)

            # fp8 arena for the DoubleRow S matmuls
            a8 = const.tile([128, AW], FP8)
            a8_state = {"filled": 0}

            def ensure_a8(j_hi):
                # cast xT -> fp8 in 8-tile pieces as xT groups land
                need = min(NT, max(0, j_hi))
                while a8_state["filled"] < need:
                    g = a8_state["filled"]
                    g2 = min(NT, g + 8)
                    ensure_xT(g2)
                    nc.vector.tensor_copy(a8[:, g * 128:g2 * 128],
                                          xT[:, g * 128:g2 * 128])
                    a8_state["filled"] = g2

            nc.vector.memset(a8[:, ZOFF:NEGI], 0.0)
            nc.vector.tensor_scalar(a8[:, NEGI:KILLA], ident, -240.0, None,
                                    mybir.AluOpType.mult)
            nc.vector.memset(a8[:, KILLA:AW], 0.0)
            nc.vector.tensor_scalar(a8[:, KILLA:KILLA + 128], ident, 240.0,
                                    None, mybir.AluOpType.mult)
            nc.vector.tensor_scalar(a8[:, KILLB + 128:KILLB + 256], ident,
                                    240.0, None, mybir.AluOpType.mult)

            def a8_ap(col, width, delta):
                return bass.AP(tensor=a8.tensor, offset=a8.offset + col,
                               ap=[a8.ap[0], [delta, 2], [1, width]])

            # z~ = [x @ W.T | 1]  (bf16), tiled [j within tile, 129]
            zt = const.tile([128, NT, 129], ctx_dtype)
            nc.vector.memset(zt[:, :, 128], 1.0)

            def emit_z_group(g):
                z_ps = ps_work.tile([128, 512], F32, name="z_ps", tag="pswork")
                for u in range(4):
                    j = g * 4 + u
                    nc.tensor.matmul(z_ps[:, u * 128:(u + 1) * 128],
                                     xT[:, j * 128:(j + 1) * 128], wT,
                                     start=True, stop=True)
                nc.vector.tensor_copy(
                    zt[:, g * 4:(g + 1) * 4, 0:128],
                    z_ps.rearrange("p (j c) -> p j c", c=128))

            z_state = {"emitted": 0}

            def ensure_z(j_hi):
                need = min(NT // 4, (j_hi + 3) // 4)
                while z_state["emitted"] < need:
                    emit_z_group(z_state["emitted"])
                    z_state["emitted"] += 1

            zeros128 = const.tile([128, 128], ctx_dtype)
            nc.vector.memset(zeros128, 0.0)
            dummy258 = const.tile([128, 258], ctx_dtype)
            nc.vector.memset(dummy258, 0.0)


            # prefetch a couple of xT/z groups so quad 0's pipeline starts deep
            ensure_xT(8)
            ensure_z(4)

            # ---- main loop ----
            # exp tiles span up to 3 PSUM banks (j-block groups of 3) to
            # amortize the ~352-cycle ACTIVATE overhead.  The four acc
            # accumulators pack two-per-bank: a zero matmul opens the bank's
            # accumulation group (start=True clears has_written bank-wide),
            # then every AV matmul accumulates with start=False.
            # S-matmuls are emitted one group AHEAD so they sit in front of
            # the previous group's AV matmuls in the PE FIFO — otherwise the
            # scalar engine stalls ~1us at every quad boundary (head-of-line
            # blocking behind AVs that wait on exp).
            JG = [2] + [3] * 10       # j-block group sizes per quad (sum=32)
            NB = QW // 128            # i-blocks per quad (4)
            groups = []
            for q in range(NQ):
                jb = 0
                for hi, gsz in enumerate(JG):
                    groups.append((q, jb, gsz, hi))
                    jb += gsz

            s_tiles = {}

            def emit_S(idx):
                q, jb, gsz, hi = groups[idx]
                ensure_a8(max(jb + gsz, 4 * q + 4))
                s_ps = ps_work.tile([128, QW * gsz], F32, name="s_ps",
                                    tag="pswork")
                for u in range(gsz):
                    j = jb + u
                    for h in range(2):
                        cols = q * 512 + h * 256
                        t0 = q * 4 + 2 * h
                        is_diag = t0 <= j <= t0 + 1
                        lhsT = a8_ap(j * 128, 128,
                                     (NEGI if is_diag else ZOFF) - j * 128)
                        if is_diag:
                            kb = KILLA if j == t0 else KILLB
                            rhs = a8_ap(cols, 256, kb - cols)
                        else:
                            rhs = a8_ap(cols, 256, 0)
                        nc.tensor.matmul(
                            s_ps[:, u * QW + h * 256:u * QW + h * 256 + 256],
                            lhsT, rhs, start=True, stop=True,
                            perf_mode=mybir.MatmulPerfMode.DoubleRow)
                s_tiles[idx] = s_ps

            emit_S(0)
            acc = None
            acc_slice = None
            for idx, (q, jb, gsz, hi) in enumerate(groups):
                if hi == 0:
                    emit_eii(q)
                    acc = [ps_acc.tile([128, 258], F32, name=f"acc{p}",
                                       tag="acc")
                           for p in range(NB // 2)]

                    def acc_slice(k, w=129, _acc=acc):
                        return _acc[k // 2][:, (k % 2) * 129:(k % 2) * 129 + w]

                if idx + 1 < len(groups):
                    emit_S(idx + 1)
                s_ps = s_tiles.pop(idx)
                b_sb = bwork.tile([128, QW * gsz], ctx_dtype, name="b_sb",
                                  tag="b_sb")
                nc.scalar.activation(b_sb, s_ps, mybir.ActivationFunctionType.Exp,
                                     scale=SCALE)
                ensure_z(jb + gsz)
                if hi == 0:
                    for p in range(NB // 2):
                        nc.tensor.matmul(acc[p], zeros128, dummy258,
                                         start=True, stop=False,
                                         skip_group_check=True)
                for u in range(gsz):
                    j = jb + u
                    for k in range(NB):
                        nc.tensor.matmul(
                            acc_slice(k),
                            b_sb[:, u * QW + k * 128:u * QW + (k + 1) * 128],
                            zt[:, j, :], start=False, stop=(j == NT - 1),
                            skip_group_check=True)
                if hi != len(JG) - 1:
                    continue
                # epilogue: y = acc[:, :128] / acc[:, 128] + bias; one DMA/quad
                y4 = ywork.tile([128, NB, 128], F32, name="y4", tag="y4")
                for k in range(NB):
                    R = q * NB + k
                    rtot = ywork.tile([128, 1], F32, name="rtot", tag="rtot")
                    nc.vector.tensor_tensor(rtot, acc_slice(k, 129)[:, 128:129],
                                            eii[:, R:R + 1], mybir.AluOpType.add)
                    rinv = ywork.tile([128, 1], F32, name="rinv", tag="rinv")
                    nc.vector.reciprocal(rinv, rtot)
                    nc.vector.scalar_tensor_tensor(
                        y4[:, k, :], zt[:, R, 0:128], eii[:, R:R + 1],
                        acc_slice(k, 128),
                        op0=mybir.AluOpType.mult, op1=mybir.AluOpType.add)
                    nc.vector.scalar_tensor_tensor(
                        y4[:, k, :], y4[:, k, :], rinv, bias_bc,
                        op0=mybir.AluOpType.mult, op1=mybir.AluOpType.add)
                o_view = o_d.rearrange("(t p) c -> p t c", p=128)
                nc.sync.dma_start(o_view[:, q * NB:(q + 1) * NB, :], y4)

    nc.compile()
    return nc


_NC_CACHE = {}


def _get_nc():
    if "nc" not in _NC_CACHE:
        _NC_CACHE["nc"] = _build()
    return _NC_CACHE["nc"]


def kernel(x, W, b, _trace=False):
    """x: [8, 4096, 128] f32, W: [128, 128] f32, b: [128] f32 -> [8, 4096, 128] f32."""
    nc = _get_nc()
    x = np.ascontiguousarray(np.asarray(x, dtype=np.float32))
    W = np.ascontiguousarray(np.asarray(W, dtype=np.float32))
    b = np.ascontiguousarray(np.asarray(b, dtype=np.float32))
    in_maps = [{"x": x[i], "W": W, "b": b} for i in range(B)]
    res = bass_utils.run_bass_kernel_spmd(nc, in_maps, core_ids=list(range(B)),
                                          trace=_trace)
    out = np.stack([r["out"] for r in res.results]).astype(np.float32)
    if _trace:
        return out, res
    return out



# revision 31
# speedup vs baseline: 1.1338x; 1.0595x over previous
"""Trainium2 Bass kernel for NonLinearSelfAttention.

Computes, per batch b:
    S    = x_b @ x_b.T * C**-0.5          [N, N]
    P    = softmax(S, axis=-1)
    out  = (P @ x_b) @ W.T + bias         [N, OUT]

Sharding: batch-data-parallel, one batch per NeuronCore (8 cores).

Per-core algorithm (N=4096, C=128):
  - E = exp(scale*S) is symmetric, so the tile E[J-block, A-block] computed in
    [j, i] layout is directly the lhsT needed by the P@V matmul for output
    block A — no transposes in the main loop.
  - The Linear folds through the attention: y = (E @ [z | 1]) / r + bias with
    z = x @ W.T, because (P x) W.T = P (x W.T).  The appended ones column
    produces the softmax row-sums r in per-partition layout for free
    (r_i = sum_j E[j, i] = sum_j E[i, j] by symmetry).
  - No max-subtraction needed: logits are ~N(0,1) with diagonal ~ sqrt(C)+,
    max ~ 20, exp(20) is well inside fp32 range.
"""
import numpy as np

import concourse.bass as bass
import concourse.tile as tile
from concourse.masks import make_identity
from concourse import bacc, mybir
from concourse import bass_utils

B = 8          # batches = cores
N = 4096       # sequence length
C = 128        # feature dim
OUT = 128      # linear out dim
NT = N // 128  # 32 j-tiles
QW = 512       # i-columns processed per quad-block
NQ = N // QW   # 8 quad blocks
SCALE = float(C) ** -0.5

F32 = mybir.dt.float32
F32R = mybir.dt.float32r
BF16 = mybir.dt.bfloat16


def _build(ctx_dtype=BF16):
    nc = bacc.Bacc("TRN2", target_bir_lowering=False, debug=False, num_devices=B)
    x_d = nc.dram_tensor("x", [N, C], F32, kind="ExternalInput").ap()
    w_d = nc.dram_tensor("W", [OUT, C], F32, kind="ExternalInput").ap()
    b_d = nc.dram_tensor("b", [OUT], F32, kind="ExternalInput").ap()
    o_d = nc.dram_tensor("out", [N, OUT], F32, kind="ExternalOutput").ap()

    with tile.TileContext(nc) as tc:
        with tc.tile_pool(name="const", bufs=1) as const, \
             tc.tile_pool(name="bwork", bufs=6) as bwork, \
             tc.tile_pool(name="ywork", bufs=2) as ywork, \
             tc.tile_pool(name="ps_work", bufs=2, space="PSUM") as ps_work, \
             tc.tile_pool(name="ps_acc", bufs=2, space="PSUM") as ps_acc:

            # ---- setup ----
            # x loads: few big DMAs (each InstDMACopy splits across all 16
            # SDMA slots; many small DMAs pay ~600ns serial issue each)
            x_nat = const.tile([128, NT, 128], F32)       # x tiles [j within tile, c]
            x_view = x_d.rearrange("(t p) c -> p t c", p=128)
            # small leading chunks so the cast/transpose pipeline starts early
            bounds = [0, 4, 8, 16, 24, NT]
            for lo, hi in zip(bounds, bounds[1:]):
                nc.sync.dma_start(x_nat[:, lo:hi, :], x_view[:, lo:hi, :])

            w_sb = const.tile([128, 128], F32)            # W [o, c]
            nc.sync.dma_start(w_sb, w_d)
            bias_bc = const.tile([128, 128], F32)         # bias broadcast to all partitions
            nc.sync.dma_start(bias_bc, bass.AP(tensor=b_d.tensor, offset=b_d.offset,
                                               ap=[[0, 128]] + b_d.ap))

            # cast to bf16, then PE-transpose (bf16: single matmul per tile;
            # fp32 transposes lower to 2).  PSUM->SBUF casts ride the scalar
            # engine, idle during setup.
            x_bf = const.tile([128, NT, 128], BF16)
            for lo, hi in zip(bounds, bounds[1:]):
                nc.vector.tensor_copy(x_bf[:, lo:hi, :], x_nat[:, lo:hi, :])
            w_bf = const.tile([128, 128], BF16)
            nc.vector.tensor_copy(w_bf, w_sb)

            ident = const.tile([128, 128], BF16)
            make_identity(nc, ident)
            xT = const.tile([128, N], BF16)               # [c, n]

            def emit_xT_group(g):
                t_ps = ps_work.tile([128, 512], BF16, name="t_ps", tag="pswork")
                for u in range(4):
                    nc.tensor.transpose(t_ps[:, u * 128:(u + 1) * 128],
                                        x_bf[:, g * 4 + u, :], ident)
                nc.vector.tensor_copy(xT[:, g * 512:(g + 1) * 512], t_ps)

            xT_state = {"emitted": 0}

            def ensure_xT(j_hi):
                need = min(NT // 4, max(1, (j_hi + 3) // 4))
                while xT_state["emitted"] < need:
                    emit_xT_group(xT_state["emitted"])
                    xT_state["emitted"] += 1

            ensure_xT(4)  # group 0: quad 0's rhs columns
            wt_ps = ps_work.tile([128, 512], BF16, name="t_ps", tag="pswork")
            nc.tensor.transpose(wt_ps[:, 0:128], w_bf, ident)
            wT = const.tile([128, 128], BF16)             # wT[c, o] = W[o, c]
            nc.vector.tensor_copy(wT, wt_ps[:, 0:128])

            # z~ = [x @ W.T | 1]  (bf16), tiled [j within tile, 129]
            zt = const.tile([128, NT, 129], ctx_dtype)
            nc.vector.memset(zt[:, :, 128], 1.0)

            def emit_z_group(g):
                z_ps = ps_work.tile([128, 512], F32, name="z_ps", tag="pswork")
                for u in range(4):
                    j = g * 4 + u
                    nc.tensor.matmul(z_ps[:, u * 128:(u + 1) * 128],
                                     xT[:, j * 128:(j + 1) * 128], wT,
                                     start=True, stop=True)
                nc.vector.tensor_copy(
                    zt[:, g * 4:(g + 1) * 4, 0:128],
                    z_ps.rearrange("p (j c) -> p j c", c=128))

            z_state = {"emitted": 0}

            def ensure_z(j_hi):
                need = min(NT // 4, (j_hi + 3) // 4)
                while z_state["emitted"] < need:
                    emit_z_group(z_state["emitted"])
                    z_state["emitted"] += 1

            zeros128 = const.tile([128, 128], ctx_dtype)
            nc.vector.memset(zeros128, 0.0)
            dummy258 = const.tile([128, 258], ctx_dtype)
            nc.vector.memset(dummy258, 0.0)


            # prefetch a couple of xT/z groups so quad 0's pipeline starts deep
            ensure_xT(8)
            ensure_z(4)

            # ---- main loop ----
            # exp tiles span up to 3 PSUM banks (j-block groups of 3) to
            # amortize the ~352-cycle ACTIVATE overhead.  The four acc
            # accumulators pack two-per-bank: a zero matmul opens the bank's
            # accumulation group (start=True clears has_written bank-wide),
            # then every AV matmul accumulates with start=False.
            # S-matmuls are emitted one group AHEAD so they sit in front of
            # the previous group's AV matmuls in the PE FIFO — otherwise the
            # scalar engine stalls ~1us at every quad boundary (head-of-line
            # blocking behind AVs that wait on exp).
            JG = [2] + [3] * 10       # j-block group sizes per quad (sum=32)
            NB = QW // 128            # i-blocks per quad (4)
            groups = []
            for q in range(NQ):
                jb = 0
                for hi, gsz in enumerate(JG):
                    groups.append((q, jb, gsz, hi))
                    jb += gsz

            s_tiles = {}

            def emit_S(idx):
                q, jb, gsz, hi = groups[idx]
                ensure_xT(jb + gsz)
                s_ps = ps_work.tile([128, QW * gsz], F32, name="s_ps",
                                    tag="pswork")
                for u in range(gsz):
                    j = jb + u
                    nc.tensor.matmul(s_ps[:, u * QW:(u + 1) * QW],
                                     xT[:, j * 128:(j + 1) * 128],
                                     xT[:, q * QW:(q + 1) * QW],
                                     start=True, stop=True)
                s_tiles[idx] = s_ps

            emit_S(0)
            acc = None
            acc_slice = None
            for idx, (q, jb, gsz, hi) in enumerate(groups):
                if hi == 0:
                    acc = [ps_acc.tile([128, 258], F32, name=f"acc{p}",
                                       tag="acc")
                           for p in range(NB // 2)]

                    def acc_slice(k, w=129, _acc=acc):
                        return _acc[k // 2][:, (k % 2) * 129:(k % 2) * 129 + w]

                if idx + 1 < len(groups):
                    emit_S(idx + 1)
                s_ps = s_tiles.pop(idx)
                b_sb = bwork.tile([128, QW * gsz], ctx_dtype, name="b_sb",
                                  tag="b_sb")
                nc.scalar.activation(b_sb, s_ps, mybir.ActivationFunctionType.Exp,
                                     scale=SCALE)
                ensure_z(jb + gsz)
                if hi == 0:
                    for p in range(NB // 2):
                        nc.tensor.matmul(acc[p], zeros128, dummy258,
                                         start=True, stop=False,
                                         skip_group_check=True)
                for u in range(gsz):
                    j = jb + u
                    for k in range(NB):
                        nc.tensor.matmul(
                            acc_slice(k),
                            b_sb[:, u * QW + k * 128:u * QW + (k + 1) * 128],
                            zt[:, j, :], start=False, stop=(j == NT - 1),
                            skip_group_check=True)
                if hi != len(JG) - 1:
                    continue
                # epilogue: y = acc[:, :128] / acc[:, 128] + bias; one DMA/quad
                y4 = ywork.tile([128, NB, 128], F32, name="y4", tag="y4")
                for k in range(NB):
                    rinv = ywork.tile([128, 1], F32, name="rinv", tag="rinv")
                    nc.vector.reciprocal(rinv, acc_slice(k, 129)[:, 128:129])
                    nc.vector.scalar_tensor_tensor(
                        y4[:, k, :], acc_slice(k, 128), rinv, bias_bc,
                        op0=mybir.AluOpType.mult, op1=mybir.AluOpType.add)
                o_view = o_d.rearrange("(t p) c -> p t c", p=128)
                nc.sync.dma_start(o_view[:, q * NB:(q + 1) * NB, :], y4)

    nc.compile()
    return nc


_NC_CACHE = {}


def _get_nc():
    if "nc" not in _NC_CACHE:
        _NC_CACHE["nc"] = _build()
    return _NC_CACHE["nc"]


def kernel(x, W, b, _trace=False):
    """x: [8, 4096, 128] f32, W: [128, 128] f32, b: [128] f32 -> [8, 4096, 128] f32."""
    nc = _get_nc()
    x = np.ascontiguousarray(np.asarray(x, dtype=np.float32))
    W = np.ascontiguousarray(np.asarray(W, dtype=np.float32))
    b = np.ascontiguousarray(np.asarray(b, dtype=np.float32))
    in_maps = [{"x": x[i], "W": W, "b": b} for i in range(B)]
    res = bass_utils.run_bass_kernel_spmd(nc, in_maps, core_ids=list(range(B)),
                                          trace=_trace)
    out = np.stack([r["out"] for r in res.results]).astype(np.float32)
    if _trace:
        return out, res
    return out



# revision 33
# speedup vs baseline: 1.1589x; 1.0222x over previous
"""Trainium2 Bass kernel for NonLinearSelfAttention.

Computes, per batch b:
    S    = x_b @ x_b.T * C**-0.5          [N, N]
    P    = softmax(S, axis=-1)
    out  = (P @ x_b) @ W.T + bias         [N, OUT]

Sharding: batch-data-parallel, one batch per NeuronCore (8 cores).

Per-core algorithm (N=4096, C=128):
  - E = exp(scale*S) is symmetric, so the tile E[J-block, A-block] computed in
    [j, i] layout is directly the lhsT needed by the P@V matmul for output
    block A — no transposes in the main loop.
  - The Linear folds through the attention: y = (E @ [z | 1]) / r + bias with
    z = x @ W.T, because (P x) W.T = P (x W.T).  The appended ones column
    produces the softmax row-sums r in per-partition layout for free
    (r_i = sum_j E[j, i] = sum_j E[i, j] by symmetry).
  - No max-subtraction needed: logits are ~N(0,1) with diagonal ~ sqrt(C)+,
    max ~ 20, exp(20) is well inside fp32 range.
"""
import numpy as np

import concourse.bass as bass
import concourse.tile as tile
from concourse.masks import make_identity
from concourse import bacc, mybir
from concourse import bass_utils

B = 8          # batches = cores
N = 4096       # sequence length
C = 128        # feature dim
OUT = 128      # linear out dim
NT = N // 128  # 32 j-tiles
QW = 512       # i-columns processed per quad-block
NQ = N // QW   # 8 quad blocks
SCALE = float(C) ** -0.5

F32 = mybir.dt.float32
F32R = mybir.dt.float32r
BF16 = mybir.dt.bfloat16


def _build(ctx_dtype=BF16):
    nc = bacc.Bacc("TRN2", target_bir_lowering=False, debug=False, num_devices=B)
    x_d = nc.dram_tensor("x", [N, C], F32, kind="ExternalInput").ap()
    w_d = nc.dram_tensor("W", [OUT, C], F32, kind="ExternalInput").ap()
    b_d = nc.dram_tensor("b", [OUT], F32, kind="ExternalInput").ap()
    o_d = nc.dram_tensor("out", [N, OUT], F32, kind="ExternalOutput").ap()

    with tile.TileContext(nc) as tc:
        with tc.tile_pool(name="const", bufs=1) as const, \
             tc.tile_pool(name="bwork", bufs=6) as bwork, \
             tc.tile_pool(name="ywork", bufs=2) as ywork, \
             tc.tile_pool(name="ps_work", bufs=2, space="PSUM") as ps_work, \
             tc.tile_pool(name="ps_acc", bufs=2, space="PSUM") as ps_acc:

            # ---- setup ----
            # x loads: few big DMAs (each InstDMACopy splits across all 16
            # SDMA slots; many small DMAs pay ~600ns serial issue each)
            # W/bias first: tiny DMAs that must not queue behind the x load
            w_sb = const.tile([128, 128], F32)            # W [o, c]
            nc.sync.dma_start(w_sb, w_d)
            bias_bc = const.tile([128, 128], F32)         # bias broadcast to all partitions
            nc.sync.dma_start(bias_bc, bass.AP(tensor=b_d.tensor, offset=b_d.offset,
                                               ap=[[0, 128]] + b_d.ap))

            x_nat = const.tile([128, NT, 128], F32)       # x tiles [j within tile, c]
            x_view = x_d.rearrange("(t p) c -> p t c", p=128)
            # small leading chunks so the cast/transpose pipeline starts early
            bounds = [0, 4, 8, 12, 16, 20, 24, 28, NT]
            for lo, hi in zip(bounds, bounds[1:]):
                nc.sync.dma_start(x_nat[:, lo:hi, :], x_view[:, lo:hi, :])

            # cast to bf16 LAZILY per 4-tile slab, then PE-transpose (bf16:
            # single matmul per tile).  Lazy casts keep the DVE FIFO short so
            # xT group 0 (and thus the first S matmul + exp) isn't queued
            # behind casts of the whole tensor.
            x_bf = const.tile([128, NT, 128], BF16)
            cast_state = {"done": 0}

            def ensure_cast(t_hi):
                while cast_state["done"] < min(NT, t_hi):
                    lo = cast_state["done"]
                    hi2 = min(NT, lo + 4)
                    nc.vector.tensor_copy(x_bf[:, lo:hi2, :], x_nat[:, lo:hi2, :])
                    cast_state["done"] = hi2

            w_bf = const.tile([128, 128], BF16)
            nc.vector.tensor_copy(w_bf, w_sb)

            ident = const.tile([128, 128], BF16)
            make_identity(nc, ident)
            xT = const.tile([128, N], BF16)               # [c, n]

            def emit_xT_group(g):
                ensure_cast(4 * g + 4)
                t_ps = ps_work.tile([128, 512], BF16, name="t_ps", tag="pswork")
                for u in range(4):
                    nc.tensor.transpose(t_ps[:, u * 128:(u + 1) * 128],
                                        x_bf[:, g * 4 + u, :], ident)
                nc.vector.tensor_copy(xT[:, g * 512:(g + 1) * 512], t_ps)

            xT_state = {"emitted": 0}

            def ensure_xT(j_hi):
                need = min(NT // 4, max(1, (j_hi + 3) // 4))
                while xT_state["emitted"] < need:
                    emit_xT_group(xT_state["emitted"])
                    xT_state["emitted"] += 1

            ensure_xT(4)  # group 0: quad 0's rhs columns
            wt_ps = ps_work.tile([128, 512], BF16, name="t_ps", tag="pswork")
            nc.tensor.transpose(wt_ps[:, 0:128], w_bf, ident)
            wT = const.tile([128, 128], BF16)             # wT[c, o] = W[o, c]
            nc.vector.tensor_copy(wT, wt_ps[:, 0:128])

            # z~ = [x @ W.T | 1]  (bf16), tiled [j within tile, 129]
            zt = const.tile([128, NT, 129], ctx_dtype)
            nc.vector.memset(zt[:, :, 128], 1.0)

            def emit_z_group(g):
                z_ps = ps_work.tile([128, 512], F32, name="z_ps", tag="pswork")
                for u in range(4):
                    j = g * 4 + u
                    nc.tensor.matmul(z_ps[:, u * 128:(u + 1) * 128],
                                     xT[:, j * 128:(j + 1) * 128], wT,
                                     start=True, stop=True)
                nc.vector.tensor_copy(
                    zt[:, g * 4:(g + 1) * 4, 0:128],
                    z_ps.rearrange("p (j c) -> p j c", c=128))

            z_state = {"emitted": 0}

            def ensure_z(j_hi):
                need = min(NT // 4, (j_hi + 3) // 4)
                while z_state["emitted"] < need:
                    emit_z_group(z_state["emitted"])
                    z_state["emitted"] += 1

            zeros128 = const.tile([128, 128], ctx_dtype)
            nc.vector.memset(zeros128, 0.0)
            dummy258 = const.tile([128, 258], ctx_dtype)
            nc.vector.memset(dummy258, 0.0)


            # prefetch a couple of xT/z groups so quad 0's pipeline starts deep
            ensure_xT(8)
            ensure_z(4)

            # ---- main loop ----
            # exp tiles span up to 3 PSUM banks (j-block groups of 3) to
            # amortize the ~352-cycle ACTIVATE overhead.  The four acc
            # accumulators pack two-per-bank: a zero matmul opens the bank's
            # accumulation group (start=True clears has_written bank-wide),
            # then every AV matmul accumulates with start=False.
            # S-matmuls are emitted one group AHEAD so they sit in front of
            # the previous group's AV matmuls in the PE FIFO — otherwise the
            # scalar engine stalls ~1us at every quad boundary (head-of-line
            # blocking behind AVs that wait on exp).
            JG = [2] + [3] * 10       # j-block group sizes per quad (sum=32)
            NB = QW // 128            # i-blocks per quad (4)
            groups = []
            for q in range(NQ):
                jb = 0
                for hi, gsz in enumerate(JG):
                    groups.append((q, jb, gsz, hi))
                    jb += gsz

            s_tiles = {}

            def emit_S(idx):
                q, jb, gsz, hi = groups[idx]
                ensure_xT(jb + gsz)
                s_ps = ps_work.tile([128, QW * gsz], F32, name="s_ps",
                                    tag="pswork")
                for u in range(gsz):
                    j = jb + u
                    nc.tensor.matmul(s_ps[:, u * QW:(u + 1) * QW],
                                     xT[:, j * 128:(j + 1) * 128],
                                     xT[:, q * QW:(q + 1) * QW],
                                     start=True, stop=True)
                s_tiles[idx] = s_ps

            emit_S(0)
            acc = None
            acc_slice = None
            for idx, (q, jb, gsz, hi) in enumerate(groups):
                if hi == 0:
                    acc = [ps_acc.tile([128, 258], F32, name=f"acc{p}",
                                       tag="acc")
                           for p in range(NB // 2)]

                    def acc_slice(k, w=129, _acc=acc):
                        return _acc[k // 2][:, (k % 2) * 129:(k % 2) * 129 + w]

                if idx + 1 < len(groups):
                    emit_S(idx + 1)
                s_ps = s_tiles.pop(idx)
                b_sb = bwork.tile([128, QW * gsz], ctx_dtype, name="b_sb",
                                  tag="b_sb")
                nc.scalar.activation(b_sb, s_ps, mybir.ActivationFunctionType.Exp,
                                     scale=SCALE)
                ensure_z(jb + gsz)
                if hi == 0:
                    for p in range(NB // 2):
                        nc.tensor.matmul(acc[p], zeros128, dummy258,
                                         start=True, stop=False,
                                         skip_group_check=True)
                for u in range(gsz):
                    j = jb + u
                    for k in range(NB):
                        nc.tensor.matmul(
                            acc_slice(k),
                            b_sb[:, u * QW + k * 128:u * QW + (k + 1) * 128],
                            zt[:, j, :], start=False, stop=(j == NT - 1),
                            skip_group_check=True)
                if hi != len(JG) - 1:
                    continue
                # epilogue: y = acc[:, :128] / acc[:, 128] + bias; one DMA/quad
                y4 = ywork.tile([128, NB, 128], F32, name="y4", tag="y4")
                for k in range(NB):
                    rinv = ywork.tile([128, 1], F32, name="rinv", tag="rinv")
                    nc.vector.reciprocal(rinv, acc_slice(k, 129)[:, 128:129])
                    nc.vector.scalar_tensor_tensor(
                        y4[:, k, :], acc_slice(k, 128), rinv, bias_bc,
                        op0=mybir.AluOpType.mult, op1=mybir.AluOpType.add)
                o_view = o_d.rearrange("(t p) c -> p t c", p=128)
                nc.sync.dma_start(o_view[:, q * NB:(q + 1) * NB, :], y4)

    nc.compile()
    return nc


_NC_CACHE = {}


def _get_nc():
    if "nc" not in _NC_CACHE:
        _NC_CACHE["nc"] = _build()
    return _NC_CACHE["nc"]


def kernel(x, W, b, _trace=False):
    """x: [8, 4096, 128] f32, W: [128, 128] f32, b: [128] f32 -> [8, 4096, 128] f32."""
    nc = _get_nc()
    x = np.ascontiguousarray(np.asarray(x, dtype=np.float32))
    W = np.ascontiguousarray(np.asarray(W, dtype=np.float32))
    b = np.ascontiguousarray(np.asarray(b, dtype=np.float32))
    in_maps = [{"x": x[i], "W": W, "b": b} for i in range(B)]
    res = bass_utils.run_bass_kernel_spmd(nc, in_maps, core_ids=list(range(B)),
                                          trace=_trace)
    out = np.stack([r["out"] for r in res.results]).astype(np.float32)
    if _trace:
        return out, res
    return out

